# revision 20
# baseline (speedup 1.0000x reference)
"""Trainium2 Bass kernel for nn_C3_layer (dense 5x5 VALID conv, 6->16 channels).

Full input x [32,6,512,512] f32 -> full output [32,16,508,508] f32.
Data-parallel over batch: 4 images per core across 8 NeuronCores.

Width-packed bf16 block-Toeplitz conv-as-matmul, batched for instruction
economy (TimelineSim showed v2 was bound by per-instruction fixed costs:
~630ns/DMA on the shared HWDGE ring and ~300ns/matmul on PE.SEQ -- engines
and HBM were under 35% busy):
  - All device I/O in bf16 (PSUM accumulates f32); host pre-casts, rel-err
    ~3e-3 vs the 2e-2 gate.
  - Width packing S=2: host de-interleaves x columns into
    xp[b,(h,ci,s),j] = x[b,ci,h,2j+s]. A 4-output-row block's 5x5 conv is
    3 matmuls (moving shifts d=0,1,2) with contraction k=(i,ci,s) K=96,
    psum m=(co,r,p) M=128, weights T[d][k,m]=W[co,ci,i-r,2d+s-p].
  - Pair fusion: two adjacent blocks (8 rows) share one PSUM bank
    ([128,508] f32); ONE matmul per d covers both via a 3D moving AP
    [96, 2(half), 254] -> 3 matmuls of N=508 per pair (768/core).
  - Group batching (G=4 pairs): ONE in-DMA per group loads [96, 2G, 256]
    through an overlapping-window AP (halves at 4-row stride), and ONE
    out-DMA per group writes [128, G*508] with 4KB-contiguous runs
    (y layout [b, co, (r,p), pair, (half,j)]). ~134 DMA instructions/core
    instead of 766.
  - Evac+bias split per pair: DVE does cols 0:254 (tensor_scalar_add), ACT
    does cols 254:508 (activation Identity+bias), both f32 psum -> bf16.
  - In-DMA issued by SP, out-DMA by ACT (two HWDGE rings); no gpsimd DMAs
    (SWDGE Q7 emission serializes), no halo chains.
"""

import os

import numpy as np

KK = 5    # conv kernel size
R = 4     # output rows per block
S = 2     # width packing factor
ND = 3    # number of moving-shift matmuls per block (kw+p = 2d+s)
B_PER_CORE = 4
N_CORES = 8
H = 512
W = 512
HO = H - 4
WO = W - 4
WP = W // S    # 256 packed input columns
WOP = WO // S  # 254 packed output columns
KDIM = (R + KK - 1) * 6 * S  # 96 contraction rows
MDIM = 16 * R * S            # 128 output partitions
NBLK = HO // R               # 127 blocks per image
NPAIR = NBLK // 2            # 63 paired blocks (+ 1 tail block)
GRP = int(os.environ.get("CONV_GROUP", "4"))  # pairs per DMA group
INSPLIT = bool(int(os.environ.get("CONV_INSPLIT", "1")))  # 2 in-DMAs/group

CH3 = np.array([[0, 1, 2], [1, 2, 3], [2, 3, 4], [3, 4, 5], [0, 4, 5], [0, 1, 5]])
CH4 = np.array([[0, 1, 2, 3], [1, 2, 3, 4], [2, 3, 4, 5], [0, 3, 4, 5], [0, 1, 4, 5],
                [0, 1, 2, 5], [0, 1, 3, 4], [1, 2, 4, 5], [0, 2, 3, 5]])

# stash of the last BassKernelResults (for test.py profiling)
LAST_RESULTS = None


def _build_full_kernel(w3, w4, w6):
    Wf = np.zeros((16, 6, KK, KK), dtype=np.float32)
    Wf[np.arange(6)[:, None], CH3] = w3
    Wf[(6 + np.arange(9))[:, None], CH4] = w4
    Wf[15] = w6[0]
    return Wf


def _build_toeplitz_packed(Wf):
    """T [ND, KDIM, MDIM]: T[d, i*12+ci*2+s, co*8+r*2+p] = Wf[co,ci,i-r,2d+s-p]
    for valid taps (0<=i-r<KK, 0<=2d+s-p<KK), else 0."""
    T = np.zeros((ND, KDIM, MDIM), dtype=np.float32)
    for d in range(ND):
        for s in range(S):
            for p in range(S):
                kw = S * d + s - p
                if not (0 <= kw < KK):
                    continue
                for r in range(R):
                    for kh in range(KK):
                        i = r + kh
                        for ci in range(6):
                            k = i * 12 + ci * 2 + s
                            m0 = r * 2 + p
                            T[d, k, m0::R * S] = Wf[:, ci, kh, kw]
    return T


def _window_ap(ap, dims):
    """Copy `ap` with an explicit [[stride, size], ...] access pattern
    (first dim = partitions). Allows overlapping windows a plain slice
    cannot express."""
    import bass_rust

    c = ap.copy()
    c.ap = bass_rust.VecI64Pair(dims)
    return c


def _build_bass():
    import contextlib

    import concourse.bacc as bacc
    import concourse.mybir as mybir
    import concourse.tile as tile

    f32 = mybir.dt.float32
    bf16 = mybir.dt.bfloat16
    # benchmarking only: repeat the whole conv body L times inside the NEFF
    loop_n = int(os.environ.get("CONV_BENCH_LOOP", "1"))
    in_bufs = int(os.environ.get("CONV_IN_BUFS", "6"))
    out_bufs = int(os.environ.get("CONV_OUT_BUFS", "6"))
    psum_bufs = int(os.environ.get("CONV_PSUM_BUFS", "8"))

    nc = bacc.Bacc(name="conv5x5p3")
    # x host layout: [b, (h, ci, s), j]; a group's input is an overlapping
    # window AP: partition (h_local, ci, s), free (half at 4-row stride, j).
    x = nc.dram_tensor("x", [B_PER_CORE, H * 6 * S, WP], bf16,
                       kind="ExternalInput")
    t = nc.dram_tensor("t", [ND, KDIM, MDIM], bf16, kind="ExternalInput")
    bias = nc.dram_tensor("bias", [MDIM, 1], f32, kind="ExternalInput")
    # y: [b, co, (r,p), pair, (half,j)]: a G-pair group's out-DMA writes
    # 128 partitions x G*1016B contiguous runs.
    y = nc.dram_tensor("y", [B_PER_CORE, 16, R * S, NPAIR, 2 * WOP],
                       bf16, kind="ExternalOutput")
    y2 = nc.dram_tensor("y2", [B_PER_CORE, 16, R * S, WOP], bf16,
                        kind="ExternalOutput")

    with tile.TileContext(nc) as tc:
        with (
            tc.tile_pool(name="const", bufs=1) as const_pool,
            tc.tile_pool(name="xin", bufs=in_bufs) as in_pool,
            tc.tile_pool(name="yout", bufs=out_bufs) as out_pool,
            tc.tile_pool(name="psum", bufs=psum_bufs, space="PSUM") as psum_pool,
        ):
            tw = const_pool.tile([KDIM, ND * MDIM], bf16, name="tw")
            nc.sync.dma_start(out=tw[:, :], in_=t.rearrange("d k m -> k d m"))
            bias_sb = const_pool.tile([MDIM, 1], f32, name="bias_sb")
            nc.sync.dma_start(out=bias_sb[:, :], in_=bias[:, :])

            loop_cm = (tc.For_i(0, loop_n, 1) if loop_n > 1
                       else contextlib.nullcontext())
            with loop_cm:
                _emit_body(nc, mybir, x, y, y2, tw, bias_sb,
                           in_pool, out_pool, psum_pool, bf16, f32)
    nc.finalize()
    return nc


def _emit_body(nc, mybir, x, y, y2, tw, bias_sb, in_pool, out_pool,
               psum_pool, bf16, f32):
    Ident = mybir.ActivationFunctionType.Identity

    # groups of GRP pairs; the last group of an image also carries the tail
    # block (4 rows) as one extra half.
    groups = []
    for b in range(B_PER_CORE):
        for g0 in range(0, NPAIR, GRP):
            pairs = list(range(g0, min(g0 + GRP, NPAIR)))
            groups.append((b, pairs, g0 + GRP >= NPAIR))

    for b, pairs, has_tail in groups:
        np_g = len(pairs)
        nh = 2 * np_g + (1 if has_tail else 0)
        h0 = pairs[0] * 2 * R  # first input row of the group
        xt = in_pool.tile([KDIM, 2 * GRP + 1, WP], bf16, name="xt", tag="xt")
        if INSPLIT and nh > 4:
            # two half-group loads: the first pairs' matmuls start as soon
            # as halves 0..3 land instead of waiting for the whole group
            src0 = _window_ap(x[b, h0 * 12:h0 * 12 + KDIM, :],
                              [[WP, KDIM], [R * 12 * WP, 4], [1, WP]])
            nc.sync.dma_start(out=xt[:, 0:4, :], in_=src0)
            h4 = h0 + 4 * R
            src1 = _window_ap(x[b, h4 * 12:h4 * 12 + KDIM, :],
                              [[WP, KDIM], [R * 12 * WP, nh - 4], [1, WP]])
            nc.sync.dma_start(out=xt[:, 4:nh, :], in_=src1)
        else:
            src = _window_ap(x[b, h0 * 12:h0 * 12 + KDIM, :],
                             [[WP, KDIM], [R * 12 * WP, nh], [1, WP]])
            nc.sync.dma_start(out=xt[:, 0:nh, :], in_=src)

        ot = out_pool.tile([MDIM, GRP, 2 * WOP], bf16, name="ot", tag="ot")
        # per-pair emission (matmuls then immediate evac) measured faster on
        # HW than d-major LDWEIGHTS amortization (202us vs 215us): prompt
        # evacuation keeps the PSUM-bank dependency path short.
        for u, pair in enumerate(pairs):
            ps = psum_pool.tile([MDIM, 2 * WOP], f32, name="ps", tag="ps")
            for d in range(ND):
                nc.tensor.matmul(
                    ps[:, :],
                    tw[:, d * MDIM:(d + 1) * MDIM],
                    xt[:, 2 * u:2 * u + 2, d:d + WOP],
                    start=(d == 0),
                    stop=(d == ND - 1),
                    skip_group_check=True,
                )
            nc.vector.tensor_scalar_add(
                ot[:, u, 0:WOP], ps[:, 0:WOP], bias_sb[:, :])
            nc.scalar.activation(
                ot[:, u, WOP:2 * WOP], ps[:, WOP:2 * WOP], Ident,
                bias=bias_sb[:, :])
        nc.scalar.dma_start(
            out=y[b, :, :, pairs[0]:pairs[0] + np_g, :],
            in_=ot[:, 0:np_g, :],
        )
        if has_tail:
            ps = psum_pool.tile([MDIM, 2 * WOP], f32, name="ps", tag="ps")
            ot2 = out_pool.tile([MDIM, WOP], bf16, name="ot2", tag="ot2")
            for d in range(ND):
                nc.tensor.matmul(
                    ps[:, 0:WOP],
                    tw[:, d * MDIM:(d + 1) * MDIM],
                    xt[:, nh - 1:nh, d:d + WOP],
                    start=(d == 0),
                    stop=(d == ND - 1),
                    skip_group_check=True,
                )
            nc.vector.tensor_scalar_add(
                ot2[:, :], ps[:, 0:WOP], bias_sb[:, :])
            nc.scalar.dma_start(out=y2[b, :, :, :], in_=ot2[:, :])


def _prep_in_maps(x, w3, b3, w4, b4, w6, b6):
    from ml_dtypes import bfloat16

    x = np.asarray(x, dtype=np.float32)
    # de-interleave width and flatten: xp[b, (h, ci, s), j] = x[b,ci,h,S*j+s]
    xp = np.ascontiguousarray(
        x.reshape(32, 6, H, WP, S).transpose(0, 2, 1, 4, 3)
        .reshape(32, H * 6 * S, WP).astype(bfloat16))
    Wf = _build_full_kernel(np.asarray(w3, dtype=np.float32),
                            np.asarray(w4, dtype=np.float32),
                            np.asarray(w6, dtype=np.float32))
    T = np.ascontiguousarray(_build_toeplitz_packed(Wf).astype(bfloat16))
    bias16 = np.concatenate([np.asarray(b3, dtype=np.float32),
                             np.asarray(b4, dtype=np.float32),
                             np.asarray(b6, dtype=np.float32)])
    bias_col = np.ascontiguousarray(
        np.repeat(bias16, R * S)[:, None], dtype=np.float32)  # [co*8+r*2+p, 1]
    return [
        {"x": xp[i * B_PER_CORE:(i + 1) * B_PER_CORE], "t": T,
         "bias": bias_col}
        for i in range(N_CORES)
    ]


def _assemble_output(results):
    ym = np.concatenate([r["y"] for r in results], axis=0)
    yt = np.concatenate([r["y2"] for r in results], axis=0)
    # ym [b, co, (r,p), pair, (half,j)] -> [b, co, oh, ow]
    main = (ym.reshape(32, 16, R, S, NPAIR, 2, WOP)
            .transpose(0, 1, 4, 5, 2, 6, 3)
            .reshape(32, 16, NPAIR * 2 * R, WO))
    tail = (yt.reshape(32, 16, R, S, WOP)
            .transpose(0, 1, 2, 4, 3)
            .reshape(32, 16, R, WO))
    return np.concatenate([main, tail], axis=2)


def kernel(x, w3, b3, w4, b4, w6, b6):
    global LAST_RESULTS
    from concourse.bass_utils import run_bass_kernel_spmd

    in_maps = _prep_in_maps(x, w3, b3, w4, b4, w6, b6)
    nc = _build_bass()
    res = run_bass_kernel_spmd(
        nc, in_maps, core_ids=list(range(N_CORES)),
        trace=bool(int(os.environ.get("CONV_TRACE", "0"))),
    )
    LAST_RESULTS = res
    out = _assemble_output(res.results)
    return np.ascontiguousarray(out.astype(np.float32))
